# revision 14
# baseline (speedup 1.0000x reference)
"""GNN message-passing layer (nn_ConvolutionLayer) on 8 Trainium2 NeuronCores.

Math:  out = leakyrelu(diag(1/deg) @ adj @ node @ W^T + b),  deg = adj.sum(-1)

Rewritten for the hardware as
    H1 = [node @ (128 W)^T + 1·(128 b)^T | 128]   (bias folded; the uniform
         128x scale keeps fp8 W/b out of e3m4's subnormal range and cancels
         through the 1/deg normalization because the deg column scales too)
    P  = adjT^T @ H1                              (last column of P is 128 deg)
    out = leakyrelu(P[:, :F] * (1/(128 deg)))     (leakyrelu is pos. homogeneous)

Sharding: data-parallel over batch B=16 -> 2 graphs per core on 8 cores.

Layout/schedule decisions:
  * adj is host-transposed to [m, n] (contraction on partitions: each 128x128
    block is directly a matmul stationary operand, no PE transposes) and cast
    to fp8 e3m4; node, 128*W^T, and the two bias halves (b = b_hi + b_lo,
    exact to ~6e-5) are packed into ONE fp8 tensor so a single 864ns DMA
    delivers the whole prologue.  Measured end-to-end rel err 1.357e-2 vs the
    2e-2 gate, fully deterministic.
  * Aggregation uses the fp8 adjT block as stationary and bf16 H1 [128, 129]
    as moving; the 129th H1 column (=128.0) makes scaled deg fall out of the
    same PSUM accumulation group.
  * TWO row blocks share each PSUM bank (cols 0:129 and 256:385) as one
    accumulation group: a whole graph needs only 4 banks, so both graphs' H1
    builds own spare banks and the PE never waits on a bank.  Per graph: an
    mcl-major first-half pass, then a pair-major second-half pass whose
    epilogues (strided DVE reciprocal + 2 fused ACT Lrelu(scale=1/deg))
    pipeline against the PE.
  * H1's first PSUM group is 2 chunks so the first DVE bias-add (gating
    aggregation start) is short; all adds finish before the first epilogue
    reciprocal so DVE never back-pressures the epilogue.
  * Dep-free warmup matmuls pay down the PE p-state ramp; a dummy activation
    preloads the Lrelu table (the only ACT table set used).
  * DMA issue is spread across sequencers (~1.2us of issuing-sequencer time
    per HWDGE DMA, ~1.1us Pool engine per SWDGE descriptor gen): the fused
    input rides SP, adj slabs ride Pool SWDGE, and the six output stores are
    balanced over Pool gens and the two SP issue slots.
"""

import ml_dtypes
import numpy as np

import concourse.mybir as mybir
import concourse.tile as tile
from concourse import bacc
from concourse.bass_utils import run_bass_kernel_spmd

B, N, F = 16, 1024, 128
NCORES = 8
G = B // NCORES          # graphs per core
P = 128                  # partitions / tile edge
MC = N // P              # contraction chunks per graph
NB = N // P              # output row blocks per graph
LEAKY_SLOPE = 0.01
WS = 128.0               # W/b prescale keeping fp8 e3m4 normal-range

W1 = 25                  # warmup matmuls (128 cols each)

f32 = mybir.dt.float32
bf16 = mybir.dt.bfloat16
fp8 = mybir.dt.float8e3

# packed prologue layout (cols of the [P, 2432] fp8 tensor)
ND0, NDW, NBH, NBL = 0, 2 * N, 2 * N + F, 2 * N + 2 * F

_nc_cache = None


def _build():
    nc = bacc.Bacc("TRN2", target_bir_lowering=False)

    adjt_d = nc.dram_tensor("adjt", [G, N, N], fp8, kind="ExternalInput")
    ndaux_d = nc.dram_tensor("ndaux", [P, 2 * N + 3 * F], fp8, kind="ExternalInput")
    out_d = nc.dram_tensor("out", [G, N, F], f32, kind="ExternalOutput")

    with tile.TileContext(nc) as tc:
        with (
            tc.tile_pool(name="const", bufs=1) as const,
            tc.tile_pool(name="rec", bufs=8) as rpool,
            tc.tile_pool(name="ps", bufs=8, space="PSUM") as pspool,
        ):
            # --- input DMAs, issued as early as possible -------------------
            ndaux = const.tile([P, 2 * N + 3 * F], fp8, tag="ndaux")
            nc.sync.dma_start(ndaux[:], ndaux_d[:])

            at = [
                [
                    const.tile(
                        [P, 4, N], fp8, tag=f"at_{g}_{h}", name=f"at_{g}_{h}"
                    )
                    for h in range(2)
                ]
                for g in range(G)
            ]
            for g in range(G):
                for h in range(2):
                    nc.gpsimd.dma_start(
                        at[g][h][:],
                        adjt_d[g, h * 4 * P:(h + 1) * 4 * P, :].rearrange(
                            "(mc p) n -> p mc n", p=P
                        ),
                    )

            def stat(g, mc, nb):
                return at[g][mc // 4][:, mc % 4, nb * P:(nb + 1) * P]

            # --- constants / PE+ACT priming --------------------------------
            ones_row = const.tile([1, P], fp8, tag="ones")
            nc.vector.memset(ones_row[:], 1.0)

            # preload the Lrelu table before the real epilogues need it
            act_dummy = const.tile([1, P], f32, tag="actdummy")
            nc.scalar.activation(
                act_dummy[:], ones_row[:], mybir.ActivationFunctionType.Lrelu,
                alpha=LEAKY_SLOPE,
            )

            h1 = [
                const.tile([P, MC, F + 1], bf16, tag=f"h1_{g}", name=f"h1_{g}")
                for g in range(G)
            ]
            for g in range(G):
                nc.vector.memset(h1[g][:, :, F:F + 1], WS)

            wps = pspool.tile([P, 512], f32, tag="ps", name="wps")
            for _ in range(W1):
                nc.tensor.matmul(
                    wps[:, 0:P], ones_row[:], ones_row[:], start=True, stop=True
                )

            # b broadcast to all 128 partitions, exactly: b_hi + b_lo
            bps = pspool.tile([P, 512], f32, tag="ps", name="bps")
            nc.tensor.matmul(
                bps[:, 0:F], ones_row[:], ndaux[0:1, NBH:NBH + F],
                start=True, stop=False,
            )
            nc.tensor.matmul(
                bps[:, 0:F], ones_row[:], ndaux[0:1, NBL:NBL + F],
                start=False, stop=True,
            )
            b_bc = const.tile([P, F], f32, tag="bbc")
            nc.vector.tensor_copy(b_bc[:], bps[:, 0:F])

            # --- H1 = [node @ (128W)^T + 128b | 128], both graphs ----------
            groups = [(0, 0, 2), (0, 2, 4), (0, 4, 8), (1, 0, 4), (1, 4, 8)]
            htiles = []
            for g, mclo, mchi in groups:
                t = pspool.tile([P, 512], f32, tag="ps", name=f"hps_{g}_{mclo}")
                htiles.append(t)
                for j, mc in enumerate(range(mclo, mchi)):
                    nc.tensor.matmul(
                        t[:, j * F:(j + 1) * F],
                        ndaux[:, g * N + mc * P:g * N + (mc + 1) * P],
                        ndaux[:, NDW:NDW + F],
                        start=(j == 0),
                        stop=(mc == mchi - 1),
                    )
            for (g, mclo, mchi), t in zip(groups, htiles):
                k = mchi - mclo
                nc.vector.tensor_add(
                    h1[g][:, mclo:mchi, 0:F],
                    t[:, 0:k * F].rearrange("p (c f) -> p c f", c=k),
                    b_bc[:, None, :].to_broadcast((P, k, F)),
                )

            # --- aggregation: two row blocks per PSUM bank -----------------
            og = [
                const.tile([P, NB, F], f32, tag=f"og_{g}", name=f"og_{g}")
                for g in range(G)
            ]

            def mm(g, pr, ab, mc, ps_pr, start, stop):
                nb = pr * 2 + ab
                nc.tensor.matmul(
                    ps_pr[:, ab * 256:ab * 256 + F + 1],
                    stat(g, mc, nb), h1[g][:, mc, :],
                    start=start, stop=stop,
                )

            def epilogue(g, pr, ab, ps_pr):
                recip = rpool.tile([P, 1], f32, tag="recip")
                nc.vector.reciprocal(
                    recip[:], ps_pr[:, ab * 256 + F:ab * 256 + F + 1]
                )
                nc.scalar.activation(
                    og[g][:, pr * 2 + ab, :],
                    ps_pr[:, ab * 256:ab * 256 + F],
                    mybir.ActivationFunctionType.Lrelu,
                    scale=recip[:],
                    alpha=LEAKY_SLOPE,
                )

            def store(g, lo, hi, engine):
                engine.dma_start(
                    out_d[g, lo * P:(hi + 1) * P, :].rearrange(
                        "(t p) f -> p t f", p=P
                    ),
                    og[g][:, lo:hi + 1, :],
                )

            for g in range(G):
                ps = [
                    pspool.tile([P, 512], f32, tag="ps", name=f"agg_{g}_{pr}")
                    for pr in range(4)
                ]
                # first-half contraction, mcl-major over all 4 bank pairs
                for mcl in range(4):
                    for pr in range(4):
                        mm(g, pr, 0, mcl, ps[pr], start=(mcl == 0), stop=False)
                        mm(g, pr, 1, mcl, ps[pr], start=False, stop=False)
                # second-half contraction, pair-major; each block's epilogue
                # fires right after its own 4 matmuls (its PSUM columns are
                # final then, even though the pair's group stops at block b)
                for pr in range(4):
                    for ab in range(2):
                        for mcl in range(4, 8):
                            mm(g, pr, ab, mcl, ps[pr], start=False,
                               stop=(ab == 1 and mcl == 7))
                        epilogue(g, pr, ab, ps[pr])
                    if g == 0:
                        if pr == 1:
                            store(g, 0, 3, nc.gpsimd)
                        elif pr == 3:
                            store(g, 4, 7, nc.gpsimd)
                    else:
                        if pr == 0:
                            store(g, 0, 1, nc.sync)
                        elif pr == 1:
                            store(g, 2, 3, nc.gpsimd)
                        elif pr == 2:
                            store(g, 4, 5, nc.sync)
                        else:
                            store(g, 6, 7, nc.gpsimd)

    nc.compile()
    return nc


def _get_nc():
    global _nc_cache
    if _nc_cache is None:
        _nc_cache = _build()
    return _nc_cache


def kernel(node_mat, adj_mat, W, b, _trace=False, _tmpdir=None):
    node_mat = np.asarray(node_mat, dtype=np.float32)
    adj_mat = np.asarray(adj_mat, dtype=np.float32)
    W = np.asarray(W, dtype=np.float32)
    b = np.asarray(b, dtype=np.float32).reshape(F)

    node_t = np.ascontiguousarray(node_mat.transpose(0, 2, 1)).astype(
        ml_dtypes.float8_e3m4
    )  # [B, F, N]
    adj_t = np.ascontiguousarray(adj_mat.transpose(0, 2, 1)).astype(
        ml_dtypes.float8_e3m4
    )  # [B, N(m), N(n)]

    bs = b * WS
    b_hi = bs.astype(ml_dtypes.float8_e3m4)
    b_lo = (bs - b_hi.astype(np.float32)).astype(ml_dtypes.float8_e3m4)
    wq = (W.T * WS).astype(ml_dtypes.float8_e3m4)  # [F_in, F_out]

    nc = _get_nc()
    in_maps = []
    for c in range(NCORES):
        ndaux = np.zeros((P, 2 * N + 3 * F), dtype=ml_dtypes.float8_e3m4)
        # node for this core's two graphs: [F, G, N] -> cols g*N + n
        ndaux[:, 0:2 * N] = node_t[c * G:(c + 1) * G].transpose(1, 0, 2).reshape(
            P, 2 * N
        )
        ndaux[:, NDW:NDW + F] = wq
        ndaux[0, NBH:NBH + F] = b_hi
        ndaux[0, NBL:NBL + F] = b_lo
        in_maps.append(
            {"adjt": adj_t[c * G:(c + 1) * G], "ndaux": ndaux}
        )
    r = run_bass_kernel_spmd(
        nc, in_maps, core_ids=list(range(NCORES)), trace=_trace, tmpdir=_tmpdir
    )
    out = np.concatenate([r.results[c]["out"] for c in range(NCORES)], axis=0)
    if _trace:
        return out, r
    return out
